# revision 3
# baseline (speedup 1.0000x reference)
"""AttentionEXT Trainium2 kernel: 8-core SPMD, sharded over N (ext points).

Reference computation (per point n, label m):
    A = enc1(ext_fea)  [N,256];  B = enc2(lab_fea)  [M,256]
    diff = A[n]-B[m];  wei = MLP(diff) [N,M,256]; softmax over m (per n,channel)
    att[n] = sum_m softmax(wei)*diff;  out = att @ fcw.T + fcb

Algebraic restructuring used here:
  * BN(eval) folded into weights on host: w' = g*w, b' = g*b+be.
  * MLP layer 1 is linear in diff: h1 = relu(P[n] + R[m]),
      P = A@W1'.T, R = b1' - B@W1'.T          (kills the [N*M,256]@[256,32] matmul)
  * softmax sums to 1  =>  att = A - U/Z  with
      E = exp(relu(y3)) = max(exp(y3),1), Z = sum_m E, U = sum_m E*B
    (no diff materialization, no softmax normalization pass)
All tensors on device live channel-major: [channels(partitions), tokens(free)].
All constant inputs are packed into one [128, PACKF] DRAM tensor so the load
is a single DMA (single semaphore: matmul sync-wait slots are scarce).
"""

import os
import sys

sys.path.insert(0, "/opt/trn_rl_repo")

import numpy as np
from concourse import bass, bacc, mybir
from concourse import tile
from concourse.bass_utils import run_bass_kernel_spmd

N, M, D_IN, H1, D, OUT_C = 2048, 128, 352, 512, 256, 13
NCORES = 8
NS = N // NCORES  # 256 ext points per core
KIN = 384  # 352 padded to 3*128
NCH = 32  # points per outer chunk
NBLK = 4  # points per 512-col inner block
F32 = mybir.dt.float32
AX = mybir.AxisListType
AF = mybir.ActivationFunctionType
ALU = mybir.AluOpType

# ---- packed constant layout: name -> (free words per partition) ----
_PACK_SPEC = [
    ("w1a", 3 * H1),   # [128,3,512]
    ("w1b", 4 * D),    # [128,4,256]
    ("w2a", 3 * H1),
    ("w2b", 4 * D),
    ("mw1", 2 * 32),   # [128,2,32]
    ("mw2", 64),       # rows 0-31 valid
    ("mw3", D),        # rows 0-63 valid
    ("fcw", 2 * OUT_C),
    ("b1a", 4),
    ("b1b", 2),
    ("b2a", 4),
    ("b2b", 2),
    ("mb1", 1),        # rows 0-31
    ("mb2", 1),        # rows 0-63
    ("mb3", 2),
    ("fcb", 1),        # rows 0-12
    ("xT", 3 * NS),    # per-core shard
    ("lT", 3 * M),
]
_PACK_OFF = {}
_off = 0
for _nm, _w in _PACK_SPEC:
    _PACK_OFF[_nm] = _off
    _off += _w
PACKF = _off

_PROG_CACHE: dict = {}


def _build_program():
    nc = bacc.Bacc(None)
    pack_d = nc.declare_dram_parameter("pack", [128, PACKF], F32, isOutput=False)
    out_d = nc.declare_dram_parameter("out", [NS, OUT_C], F32, isOutput=True)

    NCHUNKS = NS // NCH  # 8
    BLKS = NCH // NBLK  # 8 inner 512-col blocks per chunk

    with tile.TileContext(nc) as tc:
        with tc.tile_pool(name="persist", bufs=1) as wp:
            pk = wp.tile([128, PACKF], F32)
            nc.sync.dma_start(pk[:], pack_d[:])

            def sl(name, rows=128):
                a, w = _PACK_OFF[name], dict(_PACK_SPEC)[name]
                return pk[:rows, a:a + w]

            w1a_s = sl("w1a").rearrange("p (k m) -> p k m", k=3)
            w1b_s = sl("w1b").rearrange("p (k m) -> p k m", k=4)
            w2a_s = sl("w2a").rearrange("p (k m) -> p k m", k=3)
            w2b_s = sl("w2b").rearrange("p (k m) -> p k m", k=4)
            mw1_s = sl("mw1").rearrange("p (k m) -> p k m", k=2)
            mw2_s = sl("mw2", 32)
            mw3_s = sl("mw3", 64)
            fcw_s = sl("fcw").rearrange("p (k m) -> p k m", k=2)
            b1a_s = sl("b1a")
            b1b_s = sl("b1b")
            b2a_s = sl("b2a")
            b2b_s = sl("b2b")
            mb1_s = sl("mb1", 32)
            mb2_s = sl("mb2", 64)
            mb3_s = sl("mb3")
            fcb_s = sl("fcb", OUT_C)
            xT_s = sl("xT").rearrange("p (k m) -> p k m", k=3)
            lT_s = sl("lT").rearrange("p (k m) -> p k m", k=3)

            # ---- encoders ----
            B1_s = wp.tile([128, 4, M], F32)  # lab hidden [512ch, 128]
            BT_s = wp.tile([128, 2, M], F32)  # B^T        [256ch, 128]
            A1_s = wp.tile([128, 4, NS], F32)  # ext hidden [512ch, 256]
            AT_s = wp.tile([128, 2, NS], F32)  # A^T        [256ch, 256]
            PT_s = wp.tile([32, NS], F32)
            RT_s = wp.tile([32, M], F32)
            ZT_s = wp.tile([128, 2, NS], F32)
            UT_s = wp.tile([128, 2, NS], F32)

            with tc.tile_pool(name="enc_psum", bufs=2, space="PSUM") as epp:
                # lab encoder (convs2 weights)
                for mt in range(4):
                    ps = epp.tile([128, M], F32, tag="encb")
                    for kt in range(3):
                        nc.tensor.matmul(
                            ps, w2a_s[:, kt, mt * 128:(mt + 1) * 128], lT_s[:, kt],
                            start=(kt == 0), stop=(kt == 2))
                    nc.scalar.activation(B1_s[:, mt], ps, AF.Relu,
                                         bias=b2a_s[:, mt:mt + 1])
                for mt in range(2):
                    ps = epp.tile([128, M], F32, tag="encb")
                    for kt in range(4):
                        nc.tensor.matmul(
                            ps, w2b_s[:, kt, mt * 128:(mt + 1) * 128], B1_s[:, kt],
                            start=(kt == 0), stop=(kt == 3))
                    nc.scalar.activation(BT_s[:, mt], ps, AF.Relu,
                                         bias=b2b_s[:, mt:mt + 1])
                # ext encoder (convs1 weights)
                for mt in range(4):
                    ps = epp.tile([128, NS], F32, tag="enca")
                    for kt in range(3):
                        nc.tensor.matmul(
                            ps, w1a_s[:, kt, mt * 128:(mt + 1) * 128], xT_s[:, kt],
                            start=(kt == 0), stop=(kt == 2))
                    nc.scalar.activation(A1_s[:, mt], ps, AF.Relu,
                                         bias=b1a_s[:, mt:mt + 1])
                for mt in range(2):
                    ps = epp.tile([128, NS], F32, tag="enca")
                    for kt in range(4):
                        nc.tensor.matmul(
                            ps, w1b_s[:, kt, mt * 128:(mt + 1) * 128], A1_s[:, kt],
                            start=(kt == 0), stop=(kt == 3))
                    nc.scalar.activation(AT_s[:, mt], ps, AF.Relu,
                                         bias=b1b_s[:, mt:mt + 1])
                # P = A@W1'.T   (PSUM [32, NS]);  R = mb1 - B@W1'.T
                ps = epp.tile([32, NS], F32, tag="encp")
                for kt in range(2):
                    nc.tensor.matmul(ps, mw1_s[:, kt], AT_s[:, kt],
                                     start=(kt == 0), stop=(kt == 1))
                nc.scalar.activation(PT_s[:], ps, AF.Identity, bias=0.0)
                ps = epp.tile([32, M], F32, tag="encp")
                for kt in range(2):
                    nc.tensor.matmul(ps, mw1_s[:, kt], BT_s[:, kt],
                                     start=(kt == 0), stop=(kt == 1))
                nc.scalar.activation(RT_s[:], ps, AF.Identity,
                                     bias=mb1_s[:], scale=-1.0)

            # ---- bf16 copies of hot-loop operands ----
            BF = mybir.dt.bfloat16
            mw2b = wp.tile([32, 64], BF)
            nc.vector.tensor_copy(mw2b[:], mw2_s)
            mw3b = wp.tile([64, D], BF)
            nc.vector.tensor_copy(mw3b[:], mw3_s)
            BTb = wp.tile([128, 2, M], BF)
            nc.vector.tensor_copy(BTb[:], BT_s[:])
            PTb = wp.tile([32, NS], BF)
            nc.vector.tensor_copy(PTb[:], PT_s[:])
            RTb = wp.tile([32, M], BF)
            nc.vector.tensor_copy(RTb[:], RT_s[:])

            # ---- hot loop over point chunks ----
            with (
                tc.tile_pool(name="work", bufs=3) as hp,
                tc.tile_pool(name="big", bufs=2) as bp,
                tc.tile_pool(name="mm_psum", bufs=2, space="PSUM") as h2pp,
                tc.tile_pool(name="y3_psum", bufs=3, space="PSUM") as y3pp,
            ):
                for c in range(NCHUNKS):
                    nsl = slice(c * NCH, (c + 1) * NCH)
                    # h1 = relu(P[n] + R[m])  [32, NCH*M] bf16
                    h1 = hp.tile([32, NCH * M], BF, tag="h1")
                    nc.vector.tensor_tensor(
                        h1[:].rearrange("p (n m) -> p n m", m=M),
                        PTb[:, nsl][:, :, None].broadcast_to((32, NCH, M)),
                        RTb[:, None, :].broadcast_to((32, NCH, M)),
                        ALU.add)
                    nc.vector.tensor_scalar_max(h1[:], h1[:], 0.0)
                    E_s = bp.tile([128, 2, NCH * M], BF, tag="E")
                    for t in range(4):  # 1024-col groups (8 points each)
                        h2s = hp.tile([64, 1024], BF, tag="h2")
                        for bb in range(2):
                            ps2 = h2pp.tile([64, 512], F32, tag="h2ps")
                            nc.tensor.matmul(
                                ps2, mw2b[:], h1[:, t * 1024 + bb * 512:
                                                 t * 1024 + (bb + 1) * 512],
                                start=True, stop=True)
                            nc.scalar.activation(h2s[:, bb * 512:(bb + 1) * 512],
                                                 ps2, AF.Relu, bias=mb2_s[:])
                        for h in range(2):
                            ps3 = y3pp.tile([128, 1024], F32, tag="y3ps")
                            for bb in range(2):
                                nc.tensor.matmul(
                                    ps3[:, bb * 512:(bb + 1) * 512],
                                    mw3b[:, h * 128:(h + 1) * 128],
                                    h2s[:, bb * 512:(bb + 1) * 512],
                                    start=True, stop=True)
                            nc.scalar.activation(
                                E_s[:, h, t * 1024:(t + 1) * 1024], ps3, AF.Exp,
                                bias=mb3_s[:, h:h + 1])
                    # E = max(E', 1);  EB = E * B[m]
                    nc.vector.tensor_scalar_max(E_s[:], E_s[:], 1.0)
                    EB_s = bp.tile([128, 2, NCH * M], BF, tag="EB")
                    for h in range(2):
                        nc.vector.tensor_tensor(
                            EB_s[:, h].rearrange("p (n m) -> p n m", m=M),
                            E_s[:, h].rearrange("p (n m) -> p n m", m=M),
                            BTb[:, h][:, None, :].broadcast_to((128, NCH, M)),
                            ALU.mult)
                    # Z/U: in-place halving tree over m down to 8, fp32 tail
                    for src in (E_s, EB_s):
                        L = M // 2
                        while L >= 8:
                            v = src[:].rearrange("p h (n m) -> p h n m", m=M)
                            nc.vector.tensor_tensor(
                                v[:, :, :, 0:L], v[:, :, :, 0:L], v[:, :, :, L:2 * L],
                                ALU.add)
                            L //= 2
                    vE = E_s[:].rearrange("p h (n m) -> p h n m", m=M)
                    vB = EB_s[:].rearrange("p h (n m) -> p h n m", m=M)
                    nc.vector.reduce_sum(ZT_s[:, :, nsl], vE[:, :, :, 0:8], axis=AX.X)
                    nc.vector.reduce_sum(UT_s[:, :, nsl], vB[:, :, :, 0:8], axis=AX.X)

            # ---- att = A - U/Z ; out = att @ fcw.T + fcb ----
            with tc.tile_pool(name="fin", bufs=1) as fp, \
                 tc.tile_pool(name="fin_psum", bufs=1, space="PSUM") as fpp:
                Zr = fp.tile([128, 2, NS], F32)
                nc.vector.reciprocal(Zr[:], ZT_s[:])
                W_s = fp.tile([128, 2, NS], F32)
                nc.vector.tensor_tensor(W_s[:], UT_s[:], Zr[:], ALU.mult)
                ATT = fp.tile([128, 2, NS], F32)
                nc.vector.tensor_tensor(ATT[:], AT_s[:], W_s[:], ALU.subtract)
                fps = fpp.tile([OUT_C, NS], F32)
                for kt in range(2):
                    nc.tensor.matmul(fps, fcw_s[:, kt], ATT[:, kt],
                                     start=(kt == 0), stop=(kt == 1))
                outT = fp.tile([OUT_C, NS], F32)
                nc.scalar.activation(outT[:], fps, AF.Identity, bias=fcb_s[:])
                nc.sync.dma_start(out_d[:].rearrange("n c -> c n"), outT[:])

    nc.finalize()
    return nc


def _fold(w, b, g, be):
    w = np.asarray(w, np.float32)
    b = np.asarray(b, np.float32)
    g = np.asarray(g, np.float32)
    be = np.asarray(be, np.float32)
    return (g[:, None] * w).astype(np.float32), (g * b + be).astype(np.float32)


def _padk(wT, k_to):  # pad contraction (row) dim with zeros
    out = np.zeros((k_to, wT.shape[1]), np.float32)
    out[: wT.shape[0]] = wT
    return out


def _pack_block(buf, name, arr, rows=128):
    """arr: [rows, w] block -> buf[:rows, off:off+w]."""
    off, w = _PACK_OFF[name], dict(_PACK_SPEC)[name]
    assert arr.shape == (rows, w), (name, arr.shape, rows, w)
    buf[:rows, off:off + w] = arr


def _kt(wT):  # [K*128? m] -> [128, K/128 * m] partition-tiled layout
    k, m = wT.shape
    return wT.reshape(k // 128, 128, m).transpose(1, 0, 2).reshape(128, -1)


def kernel(**inputs):
    if "prog" not in _PROG_CACHE:
        _PROG_CACHE["prog"] = _build_program()
    nc = _PROG_CACHE["prog"]

    f = {k: np.asarray(v, np.float32) for k, v in inputs.items()}
    w1a, b1a = _fold(f["w1a"], f["b1a"], f["g1a"], f["be1a"])
    w1b, b1b = _fold(f["w1b"], f["b1b"], f["g1b"], f["be1b"])
    w2a, b2a = _fold(f["w2a"], f["b2a"], f["g2a"], f["be2a"])
    w2b, b2b = _fold(f["w2b"], f["b2b"], f["g2b"], f["be2b"])
    mw1, mb1 = _fold(f["mw1"], f["mb1"], f["mg1"], f["mbe1"])
    mw2, mb2 = _fold(f["mw2"], f["mb2"], f["mg2"], f["mbe2"])
    mw3, mb3 = _fold(f["mw3"], f["mb3"], f["mg3"], f["mbe3"])

    base = np.zeros((128, PACKF), np.float32)
    _pack_block(base, "w1a", _kt(_padk(w1a.T, KIN)))
    _pack_block(base, "w1b", _kt(w1b.T))
    _pack_block(base, "w2a", _kt(_padk(w2a.T, KIN)))
    _pack_block(base, "w2b", _kt(w2b.T))
    _pack_block(base, "mw1", _kt(mw1.T))
    _pack_block(base, "mw2", mw2.T, rows=32)
    _pack_block(base, "mw3", mw3.T, rows=64)
    _pack_block(base, "fcw", _kt(f["fcw"].T))
    _pack_block(base, "b1a", b1a.reshape(4, 128).T)
    _pack_block(base, "b1b", b1b.reshape(2, 128).T)
    _pack_block(base, "b2a", b2a.reshape(4, 128).T)
    _pack_block(base, "b2b", b2b.reshape(2, 128).T)
    _pack_block(base, "mb1", mb1.reshape(32, 1), rows=32)
    _pack_block(base, "mb2", mb2.reshape(64, 1), rows=64)
    _pack_block(base, "mb3", mb3.reshape(2, 128).T)
    _pack_block(base, "fcb", f["fcb"].reshape(OUT_C, 1), rows=OUT_C)
    _pack_block(base, "lT", _kt(_padk(f["lab_fea"].T, KIN)))

    in_maps = []
    for i in range(NCORES):
        buf = base.copy()
        shard = f["ext_fea"][i * NS:(i + 1) * NS]
        _pack_block(buf, "xT", _kt(_padk(shard.T, KIN)))
        in_maps.append({"pack": np.ascontiguousarray(buf)})

    res = run_bass_kernel_spmd(nc, in_maps, core_ids=list(range(NCORES)),
                               tmpdir=os.environ.get("KERNEL_TRACE_DIR"))
    _PROG_CACHE["last_res"] = res
    return np.concatenate([res.results[i]["out"] for i in range(NCORES)], axis=0)


if __name__ == "__main__":
    pass

